# revision 55
# baseline (speedup 1.0000x reference)
"""AudioGPT dense-transformer kernel for 8 Trainium2 NeuronCores.

Sharding: data-parallel over batch (2 groups of 4 cores) x tensor-parallel
over attention heads (4 heads/core). Per layer each core computes q/k/v and
attention for its heads over its batch's full sequence, normalizes with a
deferred softmax denominator (ones-column appended to V), AllGathers the
per-head attention outputs (bf16, chunked over t and overlapped with the
remaining attention work) within its 4-core group, and computes the output
projection fully (replicated) so no AllReduce is needed.

Trunk matmuls run in float32r (fp32 with the low 12 mantissa bits rounded
off, full-rate on the PE systolic array); the gathered attention outputs and
the projection/unembed run in bf16. ALL shipped tensors are bf16 (per-exec
host->NEFF input copies cost ~3us/MB through the axon relay, so input bytes
are precious); qk/v weights are upconverted to f32r on the idle Pool engine.
Core-replicated tensors (Wproj, pos, Wemb, Wout, biases) are baked into the
NEFF as Const data - loaded once at model load instead of copied per exec -
leaving only the per-core tensors (x batch, q/k/v head slices) as inputs.

Pipelining: the projection of t-chunk t-1 is interleaved 2-matmuls-at-a-time
into the (ACT/exp-bound) attention st-loop of chunk t, so the PE's in-order
queue has proj work to fill the per-iteration exp bubbles; the final
LayerNorm is fused with the unembed (each ln chunk feeds the unembed PSUM
accumulators immediately) and its elementwise chain is split across the DVE
and Pool engines.

Layouts: activations live transposed in SBUF as [c (128-partition chunks), t]
so every matmul's operands are produced in exactly the layout the next one
consumes - no on-chip transposes anywhere:
  h^T[c,t] -> qk^T = W^T@h^T (lhsT=W natural) -> scores^T[s,t]
  (lhsT=k^T slice, rhs=q^T slice, 2 heads packed in the 128-row PE array,
  both heads' scores packed into one 2-bank PSUM tile for a single wide exp)
  -> exp on ACT -> y~^T = [v|1]^T @ P (lhsT=v in [s,d] layout, produced
  directly by v = h@Wv with lhsT=h^T) -> AllGather -> proj -> h^T.
"""

import numpy as np
import ml_dtypes

import concourse.bass as bass
from concourse import bacc
import concourse.mybir as mybir
import concourse.tile as tile
from concourse.bass_utils import run_bass_kernel_spmd

# model dims (hardcoded per contract)
B, T, V, C, H, L, DH = 2, 2048, 256, 1024, 16, 8, 64
N_CORES = 8
TPG = 4            # tensor-parallel group size (heads)
HPC = H // TPG     # heads per core = 4
DLOC = HPC * DH    # local head dims = 256
P = 128
KO = C // P        # 8 c-chunks
TCH = 512          # t-chunk (matmul moving free dim)
NTC = T // TCH     # 4
NST = T // P       # 16 s-tiles
F32 = mybir.dt.float32
F32R = mybir.dt.float32r
BF16 = mybir.dt.bfloat16
AF = mybir.ActivationFunctionType

_CACHED = {}


def _round_f32r(x):
    bits = np.ascontiguousarray(x, np.float32).view(np.uint32)
    return ((bits.astype(np.int64) + 0x800).astype(np.uint32) & np.uint32(0xFFFFF000)).view(np.float32)


def _bf16(x):
    return np.ascontiguousarray(x, np.float32).astype(ml_dtypes.bfloat16)


def _build(consts, layer_reps=1, ag_mode="chunked", interleave=True):
    nc = bacc.Bacc("TRN2", target_bir_lowering=False, num_devices=N_CORES)

    # ---- per-core external inputs (distinct per core: batch / head slice).
    #      Inputs are re-copied into the NEFF's space on EVERY execution
    #      (~3us/MB through the axon relay), so everything replicated across
    #      cores is baked into the NEFF as Const data instead (loaded once
    #      at model-load time). ----
    xT_d = nc.dram_tensor("xT", [V, T], BF16, kind="ExternalInput")
    Wqk_d = nc.dram_tensor("Wqk", [L, C, 2 * DLOC], BF16, kind="ExternalInput")
    bqk_d = nc.dram_tensor("bqk", [L, P, 4], F32, kind="ExternalInput")
    Wv_d = nc.dram_tensor("Wv", [L, C, DLOC], BF16, kind="ExternalInput")
    bv_d = nc.dram_tensor("bv", [L, 1, DLOC], F32R, kind="ExternalInput")
    outT_d = nc.dram_tensor("outT", [V, T], F32, kind="ExternalOutput")

    # ---- replicated tensors baked into the NEFF ----
    Wemb_d = nc.inline_tensor(consts["Wemb"], "cWemb")
    posT_d = nc.inline_tensor(consts["posT"], "cposT")
    Wproj_d = nc.inline_tensor(consts["Wproj"], "cWproj")
    bproj_d = nc.inline_tensor(consts["bproj"], "cbproj")
    gamma_d = nc.inline_tensor(consts["gamma"], "cgamma")
    beta_d = nc.inline_tensor(consts["beta"], "cbeta")
    Wout_d = nc.inline_tensor(consts["Wout"], "cWout")
    bout_d = nc.inline_tensor(consts["bout"], "cbout")

    # collective bounce buffers, chunked over t (double-buffered across layers)
    ag_in = [
        nc.dram_tensor(f"ag_in{i}", [NTC, DLOC, TCH], BF16) for i in range(2)
    ]
    ag_out = [
        nc.dram_tensor(f"ag_out{i}", [NTC, C, TCH], BF16) for i in range(2)
    ]
    groups = [[0, 1, 2, 3], [4, 5, 6, 7]]

    with tile.TileContext(nc) as tc:
        with (
            tc.tile_pool(name="big", bufs=1) as big,
            tc.tile_pool(name="wp", bufs=2) as wp,
            tc.tile_pool(name="wc", bufs=1) as wc,
            tc.tile_pool(name="small", bufs=1) as small,
            tc.tile_pool(name="stg", bufs=2) as stg,
            tc.tile_pool(name="pmm", bufs=2, space="PSUM") as pmm,
            tc.tile_pool(name="psc", bufs=2, space="PSUM") as psc,
            tc.tile_pool(name="py", bufs=1, space="PSUM") as py,
        ):
            # ---- constants ----
            ones_f = small.tile([P, P], F32, tag="ones_f")
            nc.vector.memset(ones_f[:], 1.0)
            ones = small.tile([P, P], F32R, tag="ones")
            nc.vector.tensor_copy(out=ones[:], in_=ones_f[:])

            gamma_s = small.tile([P, KO], F32, tag="gamma")
            nc.sync.dma_start(gamma_s[:], gamma_d[:])
            beta_s = small.tile([P, KO], F32, tag="beta")
            nc.sync.dma_start(beta_s[:], beta_d[:])
            bout_s = small.tile([P, V // P], F32, tag="bout")
            nc.sync.dma_start(bout_s[:], bout_d[:])

            # ---- embedding: h^T = Wemb^T @ x^T (+bemb +pos^T) ----
            xk = []
            for ki in range(2):
                xt = big.tile([P, T], BF16, tag=f"qk{ki}")
                nc.sync.dma_start(xt[:], xT_d[ki * P:(ki + 1) * P, :])
                xk.append(xt)
            wemb_s = wc.tile([P, 2, C], BF16, tag="wc")
            nc.sync.dma_start(
                wemb_s[:], Wemb_d[:].rearrange("(a p) c -> p a c", p=P)
            )
            h_tiles = []
            for co in range(KO):
                ht = big.tile([P, T], F32R, tag=f"h{co}")
                h_tiles.append(ht)
                for tt in range(NTC // 2):
                    wsl = slice(tt * 2 * TCH, (tt + 1) * 2 * TCH)
                    ps = psc.tile([P, 2 * TCH], F32, tag="sc")
                    for hf in range(2):
                        hsl = slice(tt * 2 * TCH + hf * TCH,
                                    tt * 2 * TCH + (hf + 1) * TCH)
                        for ki in range(2):
                            nc.tensor.matmul(
                                ps[:, hf * TCH:(hf + 1) * TCH],
                                lhsT=wemb_s[:, ki, co * P:(co + 1) * P],
                                rhs=xk[ki][:, hsl],
                                start=(ki == 0),
                                stop=(ki == 1),
                                skip_group_check=True,
                            )
                    pos_t = stg.tile([P, 2 * TCH], BF16, tag="pt")
                    nc.sync.dma_start(pos_t[:], posT_d[co * P:(co + 1) * P, wsl])
                    # posT has b_emb folded in on the host
                    nc.vector.tensor_add(out=ht[:, wsl], in0=ps[:], in1=pos_t[:])

            # ---- transformer layers ----
            for l in [ll for _ in range(layer_reps) for ll in range(L)]:
                # --- q,k in transposed layout: qk^T[c',t] ---
                bqk_s = small.tile([P, 4], F32, tag=f"bqk{l % 2}")
                nc.sync.dma_start(bqk_s[:], bqk_d[l])
                qk_tiles = []
                for co in range(4):
                    # ship bf16, upconvert to f32r on the (idle) Pool engine
                    wtb = stg.tile([P, KO, P], BF16, tag="pt")
                    nc.sync.dma_start(
                        wtb[:],
                        Wqk_d[l, :, co * P:(co + 1) * P].rearrange(
                            "(ko p) m -> p ko m", p=P
                        ),
                    )
                    wt = wp.tile([P, KO, P], F32R, tag="w")
                    nc.gpsimd.tensor_copy(out=wt[:], in_=wtb[:])
                    qt = big.tile([P, T], F32R, tag=f"qk{co}")
                    qk_tiles.append(qt)
                    for tt in range(NTC // 2):
                        wsl = slice(tt * 2 * TCH, (tt + 1) * 2 * TCH)
                        ps = psc.tile([P, 2 * TCH], F32, tag="sc")
                        for hf in range(2):
                            hsl = slice(tt * 2 * TCH + hf * TCH,
                                        tt * 2 * TCH + (hf + 1) * TCH)
                            for ki in range(KO):
                                nc.tensor.matmul(
                                    ps[:, hf * TCH:(hf + 1) * TCH],
                                    lhsT=wt[:, ki, :],
                                    rhs=h_tiles[ki][:, hsl],
                                    start=(ki == 0),
                                    stop=(ki == KO - 1),
                                    skip_group_check=True,
                                )
                        nc.vector.tensor_scalar_add(
                            qt[:, wsl], ps[:], bqk_s[:, co:co + 1]
                        )

                # --- v in [s, d] layout (direct matmul), +bv, ones col ---
                wvb = stg.tile([P, KO, DLOC], BF16, tag="pt")
                nc.sync.dma_start(
                    wvb[:], Wv_d[l].rearrange("(ko p) m -> p ko m", p=P)
                )
                wv_s = wc.tile([P, KO, DLOC], F32R, tag="wc")
                nc.gpsimd.tensor_copy(out=wv_s[:], in_=wvb[:])
                bv_row = small.tile([P, DLOC], F32R, tag="bvrow")
                nc.sync.dma_start(bv_row[0:1, :], bv_d[l])
                pbv = pmm.tile([P, TCH], F32, tag="mm")
                nc.tensor.matmul(
                    pbv[:, :DLOC],
                    lhsT=ones[0:1, :],
                    rhs=bv_row[0:1, :],
                    start=True,
                    stop=True,
                )
                bvb = small.tile([P, DLOC], F32, tag="bvb")
                nc.vector.tensor_copy(out=bvb[:], in_=pbv[:, :DLOC])

                vt = big.tile([P, NST, HPC, DH + 1], F32R, tag="v")
                nc.vector.tensor_copy(
                    out=vt[:, :, :, DH:DH + 1],
                    in_=ones[:, 0:NST * HPC].rearrange(
                        "p (s h) -> p s h", s=NST
                    ).unsqueeze(3),
                )
                for st in range(NST):
                    ps = pmm.tile([P, TCH], F32, tag="mm")
                    for ki in range(KO):
                        nc.tensor.matmul(
                            ps[:, :DLOC],
                            lhsT=h_tiles[ki][:, st * P:(st + 1) * P],
                            rhs=wv_s[:, ki, :],
                            start=(ki == 0),
                            stop=(ki == KO - 1),
                        )
                    nc.vector.tensor_add(
                        out=vt[:, st, :, 0:DH],
                        in0=ps[:, :DLOC].rearrange("p (h d) -> p h d", h=HPC),
                        in1=bvb[:].rearrange("p (h d) -> p h d", h=HPC),
                    )

                # --- projection weights prefetched before attention ---
                bproj_s = small.tile([P, KO], F32, tag=f"bproj{l % 2}")
                nc.sync.dma_start(bproj_s[:], bproj_d[l])
                wpj = wc.tile([P, KO, C], BF16, tag="wc")
                nc.sync.dma_start(
                    wpj[:], Wproj_d[l].rearrange("(ko p) m -> p ko m", p=P)
                )
                new_h = [
                    big.tile([P, T], F32R, tag=f"h{co}", name=f"nh{co}")
                    for co in range(KO)
                ]

                def proj_mms(t, l=l, bproj_s=bproj_s, wpj=wpj, new_h=new_h):
                    """Generator: proj matmuls for chunk t, 2 per yield, so
                    they slot into the ACT-bound attention st-loop bubbles."""
                    gys = []
                    for ki in range(KO):
                        gt = stg.tile([P, TCH], BF16, tag=f"gy{ki}", bufs=1)
                        gys.append(gt)
                        if ag_mode == "chunked":
                            src = ag_out[l % 2][t, ki * P:(ki + 1) * P, :]
                        else:  # timing-only: skip the collective entirely
                            src = ag_in[l % 2][t, (ki % 2) * P:(ki % 2 + 1) * P, :]
                        nc.sync.dma_start(gt[:], src)
                    n_since_yield = 0
                    for co in range(KO):
                        ps = pmm.tile([P, TCH], F32, tag="mm")
                        for ki in range(KO):
                            nc.tensor.matmul(
                                ps[:],
                                lhsT=wpj[:, ki, co * P:(co + 1) * P],
                                rhs=gys[ki][:],
                                start=(ki == 0),
                                stop=(ki == KO - 1),
                            )
                            n_since_yield += 1
                            if n_since_yield == 2:
                                n_since_yield = 0
                                yield
                        nc.vector.tensor_scalar_add(
                            new_h[co][:, t * TCH:(t + 1) * TCH],
                            ps[:],
                            bproj_s[:, co:co + 1],
                        )

                def drain(gen):
                    if gen is not None:
                        for _ in gen:
                            pass

                # --- attention per t-chunk; AllGather issued per chunk;
                #     proj of chunk t-1 interleaved into the st-loop bubbles ---
                feed = None  # active proj generator
                for t in range(NTC):
                    tsl = slice(t * TCH, (t + 1) * TCH)
                    for hp in range(2):  # head pair (2hp, 2hp+1)
                        py0 = py.tile([P, TCH], F32, tag="y0")
                        py1 = py.tile([P, TCH], F32, tag="y1")
                        for st in range(NST):
                            ssl = slice(st * P, (st + 1) * P)
                            # both heads' scores into one 2-bank psum tile
                            ps01 = psc.tile([P, 2 * TCH], F32, tag="sc")
                            nc.tensor.matmul(
                                ps01[:, 0:TCH],
                                lhsT=qk_tiles[2 + hp][0:DH, ssl],
                                rhs=qk_tiles[hp][0:DH, tsl],
                                start=True,
                                stop=True,
                            )
                            nc.tensor.matmul(
                                ps01[:, TCH:2 * TCH],
                                lhsT=qk_tiles[2 + hp][DH:P, ssl],
                                rhs=qk_tiles[hp][DH:P, tsl],
                                start=True,
                                stop=True,
                            )
                            p01 = stg.tile([P, 2 * TCH], F32R, tag="pt")
                            nc.scalar.activation(
                                p01[:], ps01[:], AF.Exp, scale=0.125
                            )
                            nc.tensor.matmul(
                                py0[0:DH + 1, :],
                                lhsT=vt[:, st, 2 * hp, :],
                                rhs=p01[:, 0:TCH],
                                start=(st == 0),
                                stop=(st == NST - 1),
                                skip_group_check=True,
                            )
                            nc.tensor.matmul(
                                py1[0:DH + 1, :],
                                lhsT=vt[:, st, 2 * hp + 1, :],
                                rhs=p01[:, TCH:2 * TCH],
                                start=(st == 0),
                                stop=(st == NST - 1),
                                skip_group_check=True,
                            )
                            if feed is not None and st < NST - 2:
                                next(feed, None)
                        # finalize: y = y~ / l (bf16), staged for AllGather.
                        # Both reciprocals issue first; reserved feed yields
                        # cover the DVE->PE latency before each broadcast.
                        rrs = []
                        for j, pyt in ((0, py0), (1, py1)):
                            rr = stg.tile([P, TCH], F32R, tag="rr",
                                          name=f"rr{j}")
                            with nc.allow_low_precision(
                                reason="f32r rounding of softmax denom is intended"
                            ):
                                nc.vector.reciprocal(
                                    out=rr[DH:DH + 1, :], in_=pyt[DH:DH + 1, :]
                                )
                            rrs.append(rr)
                        for j, pyt in ((0, py0), (1, py1)):
                            h_loc = 2 * hp + j
                            if feed is not None:
                                next(feed, None)
                            prb = pmm.tile([P, TCH], F32, tag="mm")
                            nc.tensor.matmul(
                                prb[0:DH, :],
                                lhsT=ones[DH:DH + 1, 0:DH],
                                rhs=rrs[j][DH:DH + 1, :],
                                start=True,
                                stop=True,
                            )
                            rb = stg.tile([P, TCH], F32, tag="rbf", bufs=1)
                            nc.vector.tensor_copy(out=rb[0:DH, :], in_=prb[0:DH, :])
                            yst = stg.tile([P, TCH], BF16, tag="yst")
                            nc.vector.tensor_mul(
                                out=yst[0:DH, :], in0=pyt[0:DH, :], in1=rb[0:DH, :]
                            )
                            nc.sync.dma_start(
                                ag_in[l % 2][t, h_loc * DH:(h_loc + 1) * DH, :],
                                yst[0:DH, :],
                            )
                    if ag_mode == "chunked":
                        nc.gpsimd.collective_compute(
                            "AllGather",
                            mybir.AluOpType.bypass,
                            replica_groups=groups,
                            ins=[ag_in[l % 2][t].opt()],
                            outs=[ag_out[l % 2][t].opt()],
                        )
                    if interleave:
                        drain(feed)
                        feed = proj_mms(t)  # starts with the gys DMAs

                # --- projection chunks not already interleaved above ---
                if interleave:
                    drain(feed)
                    feed = None
                else:
                    for t in range(NTC):
                        drain(proj_mms(t))
                h_tiles = new_h

            # ---- final LayerNorm over c (partition-dim via ones-matmuls),
            #      fused with the unembed per 1024-col t-pair: each ln chunk
            #      feeds the unembed PSUM accumulators immediately, so ln
            #      needs only a small rotating buffer. ----
            wo_s = wc.tile([P, KO, V], BF16, tag="wc")
            nc.sync.dma_start(
                wo_s[:], Wout_d[:].rearrange("(ko p) m -> p ko m", p=P)
            )
            W2 = 2 * TCH
            for tt in range(NTC // 2):
                wsl = slice(tt * W2, (tt + 1) * W2)
                ps_s = psc.tile([P, W2], F32, tag="sc")
                for hf in range(2):
                    for ki in range(KO):
                        nc.tensor.matmul(
                            ps_s[0:1, hf * TCH:(hf + 1) * TCH],
                            lhsT=ones[:, 0:1],
                            rhs=h_tiles[ki][:, tt * W2 + hf * TCH:
                                            tt * W2 + (hf + 1) * TCH],
                            start=(ki == 0),
                            stop=(ki == KO - 1),
                            skip_group_check=True,
                        )
                ps_s2 = psc.tile([P, W2], F32, tag="sc")
                for ki in range(KO):
                    sq = stg.tile([P, W2], F32R, tag="pt")
                    nc.scalar.activation(sq[:], h_tiles[ki][:, wsl], AF.Square)
                    for hf in range(2):
                        nc.tensor.matmul(
                            ps_s2[0:1, hf * TCH:(hf + 1) * TCH],
                            lhsT=ones[:, 0:1],
                            rhs=sq[:, hf * TCH:(hf + 1) * TCH],
                            start=(ki == 0),
                            stop=(ki == KO - 1),
                            skip_group_check=True,
                        )
                mu = small.tile([P, W2], F32R, tag="mu")
                nc.vector.tensor_scalar_mul(mu[0:1, :], ps_s[0:1, :], 1.0 / C)
                ms = small.tile([P, W2], F32, tag="ms")
                nc.vector.tensor_scalar_mul(ms[0:1, :], ps_s2[0:1, :], 1.0 / C)
                var = small.tile([P, W2], F32, tag="var")
                nc.vector.tensor_mul(
                    out=var[0:1, :], in0=mu[0:1, :].bitcast(F32),
                    in1=mu[0:1, :].bitcast(F32),
                )
                nc.vector.tensor_sub(out=var[0:1, :], in0=ms[0:1, :], in1=var[0:1, :])
                nc.vector.tensor_scalar_add(var[0:1, :], var[0:1, :], 1e-5)
                nc.vector.reciprocal(out=var[0:1, :], in_=var[0:1, :])
                rstd = small.tile([P, W2], F32R, tag="rstd")
                nc.scalar.activation(rstd[0:1, :], var[0:1, :], AF.Sqrt)
                # broadcast mu, rstd to 128 partitions
                pmu = psc.tile([P, W2], F32, tag="sc")
                prs = psc.tile([P, W2], F32, tag="sc")
                for hf in range(2):
                    hs = slice(hf * TCH, (hf + 1) * TCH)
                    nc.tensor.matmul(
                        pmu[:, hs], lhsT=ones[0:1, :], rhs=mu[0:1, hs],
                        start=True, stop=True, skip_group_check=True,
                    )
                    nc.tensor.matmul(
                        prs[:, hs], lhsT=ones[0:1, :], rhs=rstd[0:1, hs],
                        start=True, stop=True, skip_group_check=True,
                    )
                mb = stg.tile([P, W2], F32, tag="rb", bufs=1)
                nc.vector.tensor_copy(out=mb[:], in_=pmu[:])
                rs = stg.tile([P, W2], F32, tag="lnd", bufs=1)
                nc.vector.tensor_copy(out=rs[:], in_=prs[:])

                # ln chunks consumed immediately by the unembed accumulators
                accums = [psc.tile([P, W2], F32, tag="sc", name=f"acc{vo}")
                          for vo in range(V // P)]
                for ki in range(KO):
                    # SBUF-only chain split between Pool (gpsimd) and DVE
                    d = stg.tile([P, W2], F32, tag=f"d{ki % 2}", bufs=1)
                    eng0 = nc.gpsimd if ki % 2 == 0 else nc.vector
                    eng1 = nc.vector if ki % 2 == 0 else nc.gpsimd
                    eng0.tensor_sub(out=d[:], in0=h_tiles[ki][:, wsl], in1=mb[:])
                    eng1.tensor_mul(out=d[:], in0=d[:], in1=rs[:])
                    lnc = stg.tile([P, W2], BF16, tag=f"ln{ki % 3}", bufs=1)
                    eng0.tensor_scalar(
                        lnc[:],
                        d[:],
                        gamma_s[:, ki:ki + 1],
                        beta_s[:, ki:ki + 1],
                        mybir.AluOpType.mult,
                        mybir.AluOpType.add,
                    )
                    for vo in range(V // P):
                        for hf in range(2):
                            hs = slice(hf * TCH, (hf + 1) * TCH)
                            nc.tensor.matmul(
                                accums[vo][:, hs],
                                lhsT=wo_s[:, ki, vo * P:(vo + 1) * P],
                                rhs=lnc[:, hs],
                                start=(ki == 0),
                                stop=(ki == KO - 1),
                                skip_group_check=True,
                            )
                for vo in range(V // P):
                    ot = stg.tile([P, W2], F32, tag="rb", bufs=1)
                    nc.vector.tensor_scalar_add(ot[:], accums[vo][:], bout_s[:, vo:vo + 1])
                    nc.sync.dma_start(outT_d[vo * P:(vo + 1) * P, wsl], ot[:])

    nc.compile()
    return nc


def _prep_consts(inputs):
    """Host-side layout of the core-replicated tensors baked into the NEFF."""
    b_emb = np.asarray(inputs["b_emb"], np.float32)
    pos = np.asarray(inputs["pos"], np.float32)
    bproj = np.asarray(inputs["bproj"], np.float32)
    gamma = np.asarray(inputs["gamma"], np.float32)
    beta = np.asarray(inputs["beta"], np.float32)
    b_out = np.asarray(inputs["b_out"], np.float32)
    return {
        "Wemb": _bf16(np.asarray(inputs["W_emb"], np.float32)),
        "posT": _bf16(pos[0].T + b_emb[:, None]),
        "Wproj": _bf16(np.asarray(inputs["Wproj"], np.float32)),
        "bproj": np.ascontiguousarray(bproj.reshape(L, KO, P).transpose(0, 2, 1)),
        "gamma": np.ascontiguousarray(gamma.reshape(KO, P).T),
        "beta": np.ascontiguousarray(beta.reshape(KO, P).T),
        "Wout": _bf16(np.asarray(inputs["W_out"], np.float32)),
        "bout": np.ascontiguousarray(b_out.reshape(V // P, P).T),
    }


def _prep_inputs(inputs):
    """Shard + lay out the per-core inputs. Returns list of 8 in_maps."""
    x = np.asarray(inputs["x"], np.float32)
    Wqkv = np.asarray(inputs["Wqkv"], np.float32)
    bqkv = np.asarray(inputs["bqkv"], np.float32)

    in_maps = []
    for core in range(N_CORES):
        b = core // TPG
        g = core % TPG
        qsl = slice(DLOC * g, DLOC * (g + 1))
        ksl = slice(C + DLOC * g, C + DLOC * (g + 1))
        vsl = slice(2 * C + DLOC * g, 2 * C + DLOC * (g + 1))
        Wqk = np.concatenate([Wqkv[:, :, qsl], Wqkv[:, :, ksl]], axis=2)
        bqk_loc = np.concatenate([bqkv[:, qsl], bqkv[:, ksl]], axis=1)  # [L,512]
        in_maps.append({
            "xT": _bf16(x[b].T),
            "Wqk": _bf16(Wqk),
            "bqk": np.ascontiguousarray(bqk_loc.reshape(L, 4, P).transpose(0, 2, 1)),
            "Wv": _bf16(Wqkv[:, :, vsl]),
            "bv": _round_f32r(np.ascontiguousarray(bqkv[:, vsl].reshape(L, 1, DLOC))),
        })
    return in_maps


def kernel(**inputs) -> np.ndarray:
    consts = _prep_consts(inputs)
    key = hash(tuple(v.tobytes() for v in consts.values()))
    if _CACHED.get("key") != key:
        _CACHED["nc"] = _build(consts)
        _CACHED["key"] = key
    nc = _CACHED["nc"]
    in_maps = _prep_inputs(inputs)
    res = run_bass_kernel_spmd(nc, in_maps, core_ids=list(range(N_CORES)))
    out = np.empty((B, T, V), np.float32)
    out[0] = res.results[0]["outT"].T
    out[1] = res.results[TPG]["outT"].T
    return out

